# revision 18
# baseline (speedup 1.0000x reference)
"""Trainium2 Bass kernel for multi-head attention (B=4, T=2048, C=1024, H=16).

Sharding: 8 cores = (batch b in 0..3) x (head-group g in 0..1, 8 heads each).
Per core: QKV projections for its 512 dims, attention for 8 heads, partial
output projection. Host sums the two per-batch partials and adds the biases
that fold out of the device computation:
  - bk drops entirely (softmax is invariant to per-query additive constants)
  - bv folds to host:   out += Wo @ bv   (softmax rows sum to 1)
  - bo added on host
  - bq is applied on-device in the Q-projection drain (scaled by 1/sqrt(dh))

v2 schedule: one unified pipeline so the scalar engine (softmax exp, the
bottleneck at ~1.1us per 128x1024 chunk) is busy from ~12us onward:
  K proj (tb-major) -> Q proj tb0 -> scores+exp for the first two pairs
  -> V proj + remaining Q proj (PE work that overlaps the exp backlog)
  -> pair-granular software pipeline: scores/exp run 2 pairs ahead of PV.
All inputs/weights in bf16 (error budget allows; halves DMA + SBUF).
PSUM: scores keep 4 banks (2-bank tiles x2); projection chains, PV pairs and
out-projection share the other 4 banks via the pv0/pv1 tags.
Score matmuls are row-tiled (K=64 head pairs run concurrently on the PE).
"""
import numpy as np
import ml_dtypes

import concourse.bass as bass
import concourse.mybir as mybir
import concourse.tile as tile
from concourse import bacc

F32 = mybir.dt.float32
BF16 = mybir.dt.bfloat16
I16 = mybir.dt.int16
AF = mybir.ActivationFunctionType
MUL = mybir.AluOpType.mult
ADD = mybir.AluOpType.add
# bf16 Schraudolph exp: bits(e^x) ~= round(x*128*log2(e) + 128*127 - 7)
A_SCH = float(128 * np.log2(np.e))
B_SCH = float(128 * 127 - 7.0)

B, T, C = 4, 2048, 1024
H, CH = 16, 64
G = 512            # dims per head-group (8 heads)
NCIN = 8           # 128-chunks of C
NCOUT = 4          # 128-chunks of G
NTB = 4            # 512-wide t blocks
NKC = 16           # 128-wide key chunks
NQB = 4            # 512-wide query blocks
SCALE = 1.0 / np.sqrt(CH)


def build_nc(debug=False):
    nc = bacc.Bacc()
    xq = nc.declare_dram_parameter("xq", [C, T], BF16, isOutput=False)
    xk = nc.declare_dram_parameter("xk", [C, T], BF16, isOutput=False)
    xv = nc.declare_dram_parameter("xv", [C, T], BF16, isOutput=False)
    wq = nc.declare_dram_parameter("wq", [C, G], BF16, isOutput=False)
    wk = nc.declare_dram_parameter("wk", [C, G], BF16, isOutput=False)
    wv = nc.declare_dram_parameter("wv", [C, G], BF16, isOutput=False)
    wo = nc.declare_dram_parameter("wo", [G, C], BF16, isOutput=False)
    bq = nc.declare_dram_parameter("bq", [128, NCOUT], F32, isOutput=False)
    out = nc.declare_dram_parameter("out", [T, C], F32, isOutput=True)

    xq_r = xq.rearrange("(c p) t -> p c t", p=128)
    xk_r = xk.rearrange("(c p) t -> p c t", p=128)
    xv_r = xv.rearrange("(c p) t -> p c t", p=128)
    wk_r = wk.rearrange("(c p) g -> p c g", p=128)
    wq_r = wq.rearrange("(c p) g -> p c g", p=128)
    wv_r = wv.rearrange("(c p) g -> p c g", p=128)

    with tile.TileContext(nc) as tc:
        with tc.tile_pool(name="persist", bufs=1) as persist, \
             tc.tile_pool(name="xs", bufs=2) as xs, \
             tc.tile_pool(name="eb", bufs=5) as eb, \
             tc.tile_pool(name="otp", bufs=2) as otp, \
             tc.tile_pool(name="dv", bufs=2) as dv, \
             tc.tile_pool(name="scp", bufs=2, space="PSUM") as scp, \
             tc.tile_pool(name="pvp", bufs=2, space="PSUM") as pvp:
            kt = [persist.tile([128, T], BF16, tag=f"kt{i}", name=f"kt{i}")
                  for i in range(NCOUT)]
            qt = [persist.tile([128, T], BF16, tag=f"qt{i}", name=f"qt{i}")
                  for i in range(NCOUT)]
            # V augmented: per-head column 64 is ones -> PV row 64 = denominator
            v_aug = persist.tile([128, NKC, 8, 65], BF16, tag="vaug")
            nc.vector.memset(v_aug[:, :, :, 64:65], 1.0)

            wk_sb = persist.tile([128, NCIN, G], BF16, tag="wk", name="wk_sb")
            wq_sb = persist.tile([128, NCIN, G], BF16, tag="wq", name="wq_sb")
            wv_sb = persist.tile([128, NCIN, G], BF16, tag="wv", name="wv_sb")
            wo_sb = persist.tile([128, NCOUT, C], BF16, tag="wo", name="wo_sb")
            bq_sb = persist.tile([128, NCOUT], F32, tag="bq", name="bq_sb")

            dma = nc.default_dma_engine.dma_start      # SP hwdge queue
            dma_gp = nc.gpsimd.dma_start               # gpsimd software DGE

            # ---------------- projection helpers (share pv psum tags) ------
            def k_proj(tb):
                xk_t = xs.tile([128, NCIN, 512], BF16, tag="xstream",
                               name="xk_t")
                for ci in range(NCIN):
                    dma(out=xk_t[:, ci, :],
                        in_=xk_r[:, ci, tb * 512:(tb + 1) * 512])
                for co in range(NCOUT):
                    ps = pvp.tile([128, 512], F32, tag=f"pv{co % 2}",
                                  name="psk")
                    for ci in range(NCIN):
                        nc.tensor.matmul(
                            ps, wk_sb[:, ci, co * 128:(co + 1) * 128],
                            xk_t[:, ci, :],
                            start=(ci == 0), stop=(ci == NCIN - 1))
                    nc.vector.tensor_copy(
                        out=kt[co][:, tb * 512:(tb + 1) * 512], in_=ps)

            def q_proj(tb):
                xq_t = xs.tile([128, NCIN, 512], BF16, tag="xstream",
                               name="xq_t")
                if tb == 0:
                    for ci in range(NCIN):
                        dma(out=xq_t[:, ci, :],
                            in_=xq_r[:, ci, tb * 512:(tb + 1) * 512])
                else:
                    dma(out=xq_t[:, :, :],
                        in_=xq_r[:, :, tb * 512:(tb + 1) * 512])
                for co in range(NCOUT):
                    ps = pvp.tile([128, 512], F32, tag=f"pv{co % 2}",
                                  name="psq")
                    for ci in range(NCIN):
                        nc.tensor.matmul(
                            ps, wq_sb[:, ci, co * 128:(co + 1) * 128],
                            xq_t[:, ci, :],
                            start=(ci == 0), stop=(ci == NCIN - 1))
                    # (ps * scale) + bq_scaled on the DVE (keeps ACT free)
                    nc.vector.tensor_scalar(
                        qt[co][:, tb * 512:(tb + 1) * 512], ps,
                        float(SCALE), bq_sb[:, co:co + 1], MUL, ADD)

            def v_proj(tb, eng=None):
                xv_t = xs.tile([128, NCIN, 512], BF16, tag="xstream",
                               name="xv_t")
                (eng or dma)(out=xv_t[:, :, :],
                             in_=xv_r[:, :, tb * 512:(tb + 1) * 512])
                for sub in range(4):
                    tcix = tb * 4 + sub
                    ps = pvp.tile([128, 512], F32, tag=f"pv{sub % 2}",
                                  name="psv")
                    for ci in range(NCIN):
                        nc.tensor.matmul(
                            ps, xv_t[:, ci, sub * 128:(sub + 1) * 128],
                            wv_sb[:, ci, :],
                            start=(ci == 0), stop=(ci == NCIN - 1))
                    nc.vector.tensor_copy(out=v_aug[:, tcix, :, 0:64], in_=ps)

            # ---------------- attention helpers ---------------------------
            def s_exp_quarter(qb, p, eh, q4):
                """Scores (row-tiled head pairs) + exp for 4 key chunks
                (quarter pair) into half-tile eh."""
                qsl = slice(qb * 512, (qb + 1) * 512)
                for k4 in range(4):
                    kc = q4 * 4 + k4
                    k8 = kc % 8
                    ksl = slice(kc * 128, (kc + 1) * 128)
                    psc = scp.tile([128, 2, 512], F32, tag="sc",
                                   name="psc")
                    nc.tensor.matmul(
                        psc[:, 0, :], kt[p][0:64, ksl],
                        qt[p][0:64, qsl], start=True, stop=True)
                    nc.tensor.matmul(
                        psc[:, 1, :], kt[p][64:128, ksl],
                        qt[p][64:128, qsl], start=True, stop=True)
                    if kc % 4 == 3:
                        # approximate exp on the DVE to offload ACT
                        nc.vector.tensor_scalar(
                            eh.bitcast(I16)[:, k8, :, :], psc,
                            A_SCH, B_SCH, MUL, ADD)
                    else:
                        nc.scalar.activation(eh[:, k8, :, :], psc, AF.Exp)

            def s_exp_half(qb, p, half):
                eh = eb.tile([128, 8, 2, 512], BF16, tag="e01", bufs=5,
                             name="e01h")
                s_exp_quarter(qb, p, eh, 2 * half)
                s_exp_quarter(qb, p, eh, 2 * half + 1)
                return eh

            def s_exp(qb, p):
                return [s_exp_half(qb, p, 0), s_exp_half(qb, p, 1)]

            def pv_drain(p, halves, ot_t):
                """P@V (ones-augmented -> row 64 is the denominator) and the
                normalized drain into ot_t[:, p, :]."""
                pv0 = pvp.tile([128, 512], F32, tag="pv0", name="pv0")
                pv1 = pvp.tile([128, 512], F32, tag="pv1", name="pv1")
                for kc in range(NKC):
                    eh = halves[kc // 8]
                    k8 = kc % 8
                    nc.tensor.matmul(
                        pv0[0:65, :], v_aug[:, kc, 2 * p, :],
                        eh[:, k8, 0, :],
                        start=(kc == 0), stop=(kc == NKC - 1))
                    nc.tensor.matmul(
                        pv1[0:65, :], v_aug[:, kc, 2 * p + 1, :],
                        eh[:, k8, 1, :],
                        start=(kc == 0), stop=(kc == NKC - 1))
                d_sb0 = dv.tile([1, 512], F32, tag="dsb0", bufs=1)
                d_sb1 = dv.tile([1, 512], F32, tag="dsb1", bufs=1)
                nc.vector.tensor_copy(out=d_sb0[0:1, :], in_=pv0[64:65, :])
                nc.vector.tensor_copy(out=d_sb1[0:1, :], in_=pv1[64:65, :])
                rec_lo = dv.tile([1, 512], F32, tag="rec_lo", bufs=1)
                rec_hi = dv.tile([1, 512], F32, tag="rec_hi", bufs=1)
                nc.vector.reciprocal_approx_fast(rec_lo[0:1, :], d_sb0[0:1, :])
                nc.vector.reciprocal_approx_fast(rec_hi[0:1, :], d_sb1[0:1, :])
                dbc_lo = dv.tile([64, 512], F32, tag="dbc_lo")
                dbc_hi = dv.tile([64, 512], F32, tag="dbc_hi")
                nc.gpsimd.partition_broadcast(dbc_lo[:, :], rec_lo[0:1, :],
                                              channels=64)
                nc.gpsimd.partition_broadcast(dbc_hi[:, :], rec_hi[0:1, :],
                                              channels=64)
                nc.vector.tensor_mul(ot_t[0:64, p, :], pv0[0:64, :],
                                     dbc_lo[:, :])
                nc.vector.tensor_mul(ot_t[64:128, p, :], pv1[0:64, :],
                                     dbc_hi[:, :])

            def oproj(qb, ot_t):
                for tcx in range(4):
                    for n in range(2):
                        pj = pvp.tile([128, 512], F32, tag=f"pv{n}",
                                      name="pj")
                        for p2 in range(NCOUT):
                            nc.tensor.matmul(
                                pj, ot_t[:, p2, tcx * 128:(tcx + 1) * 128],
                                wo_sb[:, p2, n * 512:(n + 1) * 512],
                                start=(p2 == 0), stop=(p2 == NCOUT - 1))
                        oj = dv.tile([128, 512], F32, tag="oj")
                        nc.vector.tensor_copy(out=oj, in_=pj)
                        r0 = qb * 512 + tcx * 128
                        eng = dma_gp if n == 0 else dma
                        eng(out=out[r0:r0 + 128, n * 512:(n + 1) * 512],
                            in_=oj)

            # ---------------- emission schedule ---------------------------
            for ci in range(NCIN):
                dma(out=wk_sb[:, ci, :], in_=wk_r[:, ci, :])
            k_proj(0)
            for ci in range(NCIN):
                dma(out=wq_sb[:, ci, :], in_=wq_r[:, ci, :])
            dma(out=bq_sb, in_=bq[:, :])
            q_proj(0)
            k_proj(1)

            ot_tiles = {0: otp.tile([128, NCOUT, 512], BF16, tag="ot",
                                    name="ot_t")}
            # qb0 phase h0 (kc0-7: xk tb0/tb1) emitted right after k1 so the
            # static PE order matches real DMA arrival
            qb0_halves = {p: [s_exp_half(0, p, 0)] for p in range(NCOUT)}
            k_proj(2)
            dma(out=wv_sb[:, :, :], in_=wv_r[:, :, :])
            v_proj(0)
            k_proj(3)
            v_proj(1)
            # qb0 phase h1 (kc8-15: xk tb2/tb3); pair-serial so the ACT queue
            # never blocks on a PV whose inputs are emitted later
            for p in range(NCOUT):
                qb0_halves[p].append(s_exp_half(0, p, 1))
            v_proj(2, dma_gp)
            v_proj(3, dma_gp)
            dma(out=wo_sb, in_=wo.rearrange("(c p) g -> p c g", p=128))
            for tb in range(1, NTB):
                q_proj(tb)
            for p in range(NCOUT):
                pv_drain(p, qb0_halves[p], ot_tiles[0])
            oproj(0, ot_tiles[0])

            pairs = [(qb, p) for qb in range(1, NQB) for p in range(NCOUT)]
            pending = []
            for idx, (qb, p) in enumerate(pairs):
                if p == 0:
                    ot_tiles[qb] = otp.tile([128, NCOUT, 512], BF16,
                                            tag="ot", name="ot_t")
                pending.append((qb, p, s_exp(qb, p)))
                lookahead = 1 if qb == NQB - 1 else 2
                while len(pending) > lookahead:
                    dqb, dp, halves = pending.pop(0)
                    pv_drain(dp, halves, ot_tiles[dqb])
                    if dp == NCOUT - 1:
                        oproj(dqb, ot_tiles[dqb])
            while pending:
                dqb, dp, halves = pending.pop(0)
                pv_drain(dp, halves, ot_tiles[dqb])
                if dp == NCOUT - 1:
                    oproj(dqb, ot_tiles[dqb])
    nc.finalize()
    return nc


_CACHE = {}


def _get_runner():
    """Compile once per process; return f(in_maps) -> list of out dicts."""
    if "runner" in _CACHE:
        return _CACHE["runner"]
    import jax
    from jax.sharding import Mesh, PartitionSpec
    from jax.experimental.shard_map import shard_map
    from concourse import bass2jax

    nc = build_nc()
    bass2jax.install_neuronx_cc_hook()
    in_names, out_names, out_avals, zero_shapes = [], [], [], []
    for alloc in nc.m.functions[0].allocations:
        if not isinstance(alloc, mybir.MemoryLocationSet):
            continue
        name = alloc.memorylocations[0].name
        if alloc.kind == "ExternalInput":
            if name != "partition_id":
                in_names.append(name)
        elif alloc.kind == "ExternalOutput":
            out_names.append(name)
            shape = tuple(alloc.tensor_shape)
            dtype = mybir.dt.np(alloc.dtype)
            out_avals.append(jax.core.ShapedArray(shape, dtype))
            zero_shapes.append((shape, dtype))
    n_params = len(in_names)
    all_names = tuple(in_names + out_names)
    donate = tuple(range(n_params, n_params + len(out_names)))
    has_pid = nc.partition_id_tensor is not None

    def _body(*args):
        operands = list(args)
        names = all_names
        if has_pid:
            operands.append(bass2jax.partition_id_tensor())
            names = all_names + ("partition_id",)
        outs = bass2jax._bass_exec_p.bind(
            *operands, out_avals=tuple(out_avals), in_names=names,
            out_names=tuple(out_names), lowering_input_output_aliases=(),
            sim_require_finite=False, sim_require_nnan=False, nc=nc)
        return tuple(outs)

    devices = jax.devices()[:8]
    mesh = Mesh(np.asarray(devices), ("core",))
    specs = (PartitionSpec("core"),) * (n_params + len(out_names))
    f = jax.jit(shard_map(_body, mesh=mesh, in_specs=specs,
                          out_specs=(PartitionSpec("core"),) * len(out_names),
                          check_rep=False),
                donate_argnums=donate, keep_unused=True)

    def run(in_maps):
        concat_in = [np.concatenate([m[n] for m in in_maps], axis=0)
                     for n in in_names]
        concat_zeros = [np.zeros((8 * s[0], *s[1:]), d) for s, d in zero_shapes]
        outs = f(*concat_in, *concat_zeros)
        res = []
        for c in range(8):
            res.append({name: np.asarray(outs[i]).reshape(8, *out_avals[i].shape)[c]
                        for i, name in enumerate(out_names)})
        return res

    _CACHE["runner"] = run
    _CACHE["nc"] = nc
    return run


def make_in_maps(k, q, v, Wk, bk, Wq, bq, Wv, bv, Wo, bo):
    in_maps = []
    for c in range(8):
        b, g = divmod(c, 2)
        gs, ge = g * G, (g + 1) * G
        bqs = (bq[gs:ge] * SCALE).reshape(NCOUT, 128).T
        in_maps.append({
            "xq": np.ascontiguousarray(q[b].T).astype(ml_dtypes.bfloat16),
            "xk": np.ascontiguousarray(k[b].T).astype(ml_dtypes.bfloat16),
            "xv": np.ascontiguousarray(v[b].T).astype(ml_dtypes.bfloat16),
            "wq": np.ascontiguousarray(Wq[gs:ge, :].T).astype(ml_dtypes.bfloat16),
            "wk": np.ascontiguousarray(Wk[gs:ge, :].T).astype(ml_dtypes.bfloat16),
            "wv": np.ascontiguousarray(Wv[gs:ge, :].T).astype(ml_dtypes.bfloat16),
            "wo": np.ascontiguousarray(Wo[:, gs:ge].T).astype(ml_dtypes.bfloat16),
            "bq": np.ascontiguousarray(bqs, dtype=np.float32),
        })
    return in_maps


def kernel(k, q, v, Wk, bk, Wq, bq, Wv, bv, Wo, bo):
    k = np.asarray(k, dtype=np.float32)
    q = np.asarray(q, dtype=np.float32)
    v = np.asarray(v, dtype=np.float32)
    Wk, bk = np.asarray(Wk, np.float32), np.asarray(bk, np.float32)
    Wq, bq = np.asarray(Wq, np.float32), np.asarray(bq, np.float32)
    Wv, bv = np.asarray(Wv, np.float32), np.asarray(bv, np.float32)
    Wo, bo = np.asarray(Wo, np.float32), np.asarray(bo, np.float32)

    in_maps = make_in_maps(k, q, v, Wk, bk, Wq, bq, Wv, bv, Wo, bo)
    run = _get_runner()
    res = run(in_maps)
    host_bias = (bo + Wo @ bv).astype(np.float32)
    out = np.empty((B, T, C), np.float32)
    for b in range(B):
        out[b] = res[2 * b]["out"] + res[2 * b + 1]["out"] + host_bias[None, :]
    return out


# revision 19
# speedup vs baseline: 1.2087x; 1.2087x over previous
"""Trainium2 Bass kernel for multi-head attention (B=4, T=2048, C=1024, H=16).

Sharding: 8 cores = (batch b in 0..3) x (head-group g in 0..1, 8 heads each).
Per core: QKV projections for its 512 dims, attention for 8 heads, partial
output projection. Host sums the two per-batch partials and adds the biases
that fold out of the device computation:
  - bk drops entirely (softmax is invariant to per-query additive constants)
  - bv folds to host:   out += Wo @ bv   (softmax rows sum to 1)
  - bo added on host
  - bq is applied on-device in the Q-projection drain (scaled by 1/sqrt(dh))

v2 schedule: one unified pipeline so the scalar engine (softmax exp, the
bottleneck at ~1.1us per 128x1024 chunk) is busy from ~12us onward:
  K proj (tb-major) -> Q proj tb0 -> scores+exp for the first two pairs
  -> V proj + remaining Q proj (PE work that overlaps the exp backlog)
  -> pair-granular software pipeline: scores/exp run 2 pairs ahead of PV.
All inputs/weights in bf16 (error budget allows; halves DMA + SBUF).
PSUM: scores keep 4 banks (2-bank tiles x2); projection chains, PV pairs and
out-projection share the other 4 banks via the pv0/pv1 tags.
Score matmuls are row-tiled (K=64 head pairs run concurrently on the PE).
"""
import numpy as np
import ml_dtypes

import concourse.bass as bass
import concourse.mybir as mybir
import concourse.tile as tile
from concourse import bacc

F32 = mybir.dt.float32
BF16 = mybir.dt.bfloat16
I16 = mybir.dt.int16
AF = mybir.ActivationFunctionType
MUL = mybir.AluOpType.mult
ADD = mybir.AluOpType.add
# bf16 Schraudolph exp: bits(e^x) ~= round(x*128*log2(e) + 128*127 - 7)
A_SCH = float(128 * np.log2(np.e))
B_SCH = float(128 * 127 - 7.0)

B, T, C = 4, 2048, 1024
H, CH = 16, 64
G = 512            # dims per head-group (8 heads)
NCIN = 8           # 128-chunks of C
NCOUT = 4          # 128-chunks of G
NTB = 4            # 512-wide t blocks
NKC = 16           # 128-wide key chunks
NQB = 4            # 512-wide query blocks
SCALE = 1.0 / np.sqrt(CH)


def build_nc(debug=False):
    nc = bacc.Bacc()
    xq = nc.declare_dram_parameter("xq", [C, T], BF16, isOutput=False)
    xk = nc.declare_dram_parameter("xk", [C, T], BF16, isOutput=False)
    xv = nc.declare_dram_parameter("xv", [C, T], BF16, isOutput=False)
    wq = nc.declare_dram_parameter("wq", [C, G], BF16, isOutput=False)
    wk = nc.declare_dram_parameter("wk", [C, G], BF16, isOutput=False)
    wv = nc.declare_dram_parameter("wv", [C, G], BF16, isOutput=False)
    wo = nc.declare_dram_parameter("wo", [G, C], BF16, isOutput=False)
    bq = nc.declare_dram_parameter("bq", [128, NCOUT], F32, isOutput=False)
    out = nc.declare_dram_parameter("out", [T, C], F32, isOutput=True)

    xq_r = xq.rearrange("(c p) t -> p c t", p=128)
    xk_r = xk.rearrange("(c p) t -> p c t", p=128)
    xv_r = xv.rearrange("(c p) t -> p c t", p=128)
    wk_r = wk.rearrange("(c p) g -> p c g", p=128)
    wq_r = wq.rearrange("(c p) g -> p c g", p=128)
    wv_r = wv.rearrange("(c p) g -> p c g", p=128)

    with tile.TileContext(nc) as tc:
        with tc.tile_pool(name="persist", bufs=1) as persist, \
             tc.tile_pool(name="xs", bufs=2) as xs, \
             tc.tile_pool(name="eb", bufs=5) as eb, \
             tc.tile_pool(name="otp", bufs=2) as otp, \
             tc.tile_pool(name="dv", bufs=2) as dv, \
             tc.tile_pool(name="scp", bufs=2, space="PSUM") as scp, \
             tc.tile_pool(name="pvp", bufs=2, space="PSUM") as pvp:
            kt = [persist.tile([128, T], BF16, tag=f"kt{i}", name=f"kt{i}")
                  for i in range(NCOUT)]
            qt = [persist.tile([128, T], BF16, tag=f"qt{i}", name=f"qt{i}")
                  for i in range(NCOUT)]
            # V augmented: per-head column 64 is ones -> PV row 64 = denominator
            v_aug = persist.tile([128, NKC, 8, 65], BF16, tag="vaug")
            nc.vector.memset(v_aug[:, :, :, 64:65], 1.0)

            wk_sb = persist.tile([128, NCIN, G], BF16, tag="wk", name="wk_sb")
            wq_sb = persist.tile([128, NCIN, G], BF16, tag="wq", name="wq_sb")
            wv_sb = persist.tile([128, NCIN, G], BF16, tag="wv", name="wv_sb")
            wo_sb = persist.tile([128, NCOUT, C], BF16, tag="wo", name="wo_sb")
            bq_sb = persist.tile([128, NCOUT], F32, tag="bq", name="bq_sb")

            dma = nc.default_dma_engine.dma_start      # SP hwdge queue
            dma_gp = nc.gpsimd.dma_start               # gpsimd software DGE

            # ---------------- projection helpers (share pv psum tags) ------
            def k_proj(tb):
                xk_t = xs.tile([128, NCIN, 512], BF16, tag="xstream",
                               name="xk_t")
                for ci in range(NCIN):
                    dma(out=xk_t[:, ci, :],
                        in_=xk_r[:, ci, tb * 512:(tb + 1) * 512])
                for co in range(NCOUT):
                    ps = pvp.tile([128, 512], F32, tag=f"pv{co % 2}",
                                  name="psk")
                    for ci in range(NCIN):
                        nc.tensor.matmul(
                            ps, wk_sb[:, ci, co * 128:(co + 1) * 128],
                            xk_t[:, ci, :],
                            start=(ci == 0), stop=(ci == NCIN - 1))
                    nc.vector.tensor_copy(
                        out=kt[co][:, tb * 512:(tb + 1) * 512], in_=ps)

            def q_proj(tb):
                xq_t = xs.tile([128, NCIN, 512], BF16, tag="xstream",
                               name="xq_t")
                if tb == 0:
                    for ci in range(NCIN):
                        dma(out=xq_t[:, ci, :],
                            in_=xq_r[:, ci, tb * 512:(tb + 1) * 512])
                else:
                    dma_gp(out=xq_t[:, :, :],
                           in_=xq_r[:, :, tb * 512:(tb + 1) * 512])
                for co in range(NCOUT):
                    ps = pvp.tile([128, 512], F32, tag=f"pv{co % 2}",
                                  name="psq")
                    for ci in range(NCIN):
                        nc.tensor.matmul(
                            ps, wq_sb[:, ci, co * 128:(co + 1) * 128],
                            xq_t[:, ci, :],
                            start=(ci == 0), stop=(ci == NCIN - 1))
                    # (ps * scale) + bq_scaled on the DVE (keeps ACT free)
                    nc.vector.tensor_scalar(
                        qt[co][:, tb * 512:(tb + 1) * 512], ps,
                        float(SCALE), bq_sb[:, co:co + 1], MUL, ADD)

            def v_proj(tb):
                xv_t = xs.tile([128, NCIN, 512], BF16, tag="xstream",
                               name="xv_t")
                dma_gp(out=xv_t[:, :, :],
                       in_=xv_r[:, :, tb * 512:(tb + 1) * 512])
                for sub in range(4):
                    tcix = tb * 4 + sub
                    ps = pvp.tile([128, 512], F32, tag=f"pv{sub % 2}",
                                  name="psv")
                    for ci in range(NCIN):
                        nc.tensor.matmul(
                            ps, xv_t[:, ci, sub * 128:(sub + 1) * 128],
                            wv_sb[:, ci, :],
                            start=(ci == 0), stop=(ci == NCIN - 1))
                    nc.vector.tensor_copy(out=v_aug[:, tcix, :, 0:64], in_=ps)

            # ---------------- attention helpers ---------------------------
            def s_exp(qb, p):
                """Scores (row-tiled head pairs) + exp for one pair.
                Returns the two e01 half tiles [128, 8, 2, 512]."""
                qsl = slice(qb * 512, (qb + 1) * 512)
                halves = []
                for half in range(2):
                    eh = eb.tile([128, 8, 2, 512], BF16, tag="e01", bufs=5,
                                 name="e01h")
                    halves.append(eh)
                    for k8 in range(8):
                        kc = half * 8 + k8
                        ksl = slice(kc * 128, (kc + 1) * 128)
                        psc = scp.tile([128, 2, 512], F32, tag="sc",
                                       name="psc")
                        nc.tensor.matmul(
                            psc[:, 0, :], kt[p][0:64, ksl],
                            qt[p][0:64, qsl], start=True, stop=True)
                        nc.tensor.matmul(
                            psc[:, 1, :], kt[p][64:128, ksl],
                            qt[p][64:128, qsl], start=True, stop=True)
                        if kc % 4 == 3:
                            # approximate exp on the DVE to offload ACT
                            nc.vector.tensor_scalar(
                                eh.bitcast(I16)[:, k8, :, :], psc,
                                A_SCH, B_SCH, MUL, ADD)
                        else:
                            nc.scalar.activation(eh[:, k8, :, :], psc, AF.Exp)
                return halves

            def pv_drain(p, halves, ot_t):
                """P@V (ones-augmented -> row 64 is the denominator) and the
                normalized drain into ot_t[:, p, :]."""
                pv0 = pvp.tile([128, 512], F32, tag="pv0", name="pv0")
                pv1 = pvp.tile([128, 512], F32, tag="pv1", name="pv1")
                for kc in range(NKC):
                    eh = halves[kc // 8]
                    k8 = kc % 8
                    nc.tensor.matmul(
                        pv0[0:65, :], v_aug[:, kc, 2 * p, :],
                        eh[:, k8, 0, :],
                        start=(kc == 0), stop=(kc == NKC - 1))
                    nc.tensor.matmul(
                        pv1[0:65, :], v_aug[:, kc, 2 * p + 1, :],
                        eh[:, k8, 1, :],
                        start=(kc == 0), stop=(kc == NKC - 1))
                d_sb0 = dv.tile([1, 512], F32, tag="dsb0", bufs=1)
                d_sb1 = dv.tile([1, 512], F32, tag="dsb1", bufs=1)
                nc.vector.tensor_copy(out=d_sb0[0:1, :], in_=pv0[64:65, :])
                nc.vector.tensor_copy(out=d_sb1[0:1, :], in_=pv1[64:65, :])
                rec_lo = dv.tile([1, 512], F32, tag="rec_lo", bufs=1)
                rec_hi = dv.tile([1, 512], F32, tag="rec_hi", bufs=1)
                nc.vector.reciprocal_approx_fast(rec_lo[0:1, :], d_sb0[0:1, :])
                nc.vector.reciprocal_approx_fast(rec_hi[0:1, :], d_sb1[0:1, :])
                dbc_lo = dv.tile([64, 512], F32, tag="dbc_lo")
                dbc_hi = dv.tile([64, 512], F32, tag="dbc_hi")
                nc.gpsimd.partition_broadcast(dbc_lo[:, :], rec_lo[0:1, :],
                                              channels=64)
                nc.gpsimd.partition_broadcast(dbc_hi[:, :], rec_hi[0:1, :],
                                              channels=64)
                nc.vector.tensor_mul(ot_t[0:64, p, :], pv0[0:64, :],
                                     dbc_lo[:, :])
                nc.vector.tensor_mul(ot_t[64:128, p, :], pv1[0:64, :],
                                     dbc_hi[:, :])

            def oproj(qb, ot_t):
                for tcx in range(4):
                    for n in range(2):
                        pj = pvp.tile([128, 512], F32, tag=f"pv{n}",
                                      name="pj")
                        for p2 in range(NCOUT):
                            nc.tensor.matmul(
                                pj, ot_t[:, p2, tcx * 128:(tcx + 1) * 128],
                                wo_sb[:, p2, n * 512:(n + 1) * 512],
                                start=(p2 == 0), stop=(p2 == NCOUT - 1))
                        oj = dv.tile([128, 512], F32, tag="oj")
                        nc.vector.tensor_copy(out=oj, in_=pj)
                        r0 = qb * 512 + tcx * 128
                        dma(out=out[r0:r0 + 128, n * 512:(n + 1) * 512],
                            in_=oj)

            # ---------------- emission schedule ---------------------------
            for ci in range(NCIN):
                dma(out=wk_sb[:, ci, :], in_=wk_r[:, ci, :])
            # V-side streams on the gpsimd software DGE in parallel with SP
            dma_gp(out=wv_sb[:, :, :], in_=wv_r[:, :, :])
            k_proj(0)
            for ci in range(NCIN):
                dma(out=wq_sb[:, ci, :], in_=wq_r[:, ci, :])
            dma(out=bq_sb, in_=bq[:, :])
            q_proj(0)
            for tb in range(1, NTB):
                k_proj(tb)

            pairs = [(qb, p) for qb in range(NQB) for p in range(NCOUT)]
            ot_tiles = {}
            pending = []
            for idx, (qb, p) in enumerate(pairs):
                if p == 0:
                    ot_tiles[qb] = otp.tile([128, NCOUT, 512], BF16,
                                            tag="ot", name="ot_t")
                pending.append((qb, p, s_exp(qb, p)))
                if idx == 1:
                    # PE work to fill the exp backlog window: V proj (PV
                    # needs all of it) and the remaining Q blocks.
                    dma(out=wo_sb,
                        in_=wo.rearrange("(c p) g -> p c g", p=128))
                    for tb in range(NTB):
                        v_proj(tb)
                    for tb in range(1, NTB):
                        q_proj(tb)
                if len(pending) > 2:
                    dqb, dp, halves = pending.pop(0)
                    pv_drain(dp, halves, ot_tiles[dqb])
                    if dp == NCOUT - 1:
                        oproj(dqb, ot_tiles[dqb])
            while pending:
                dqb, dp, halves = pending.pop(0)
                pv_drain(dp, halves, ot_tiles[dqb])
                if dp == NCOUT - 1:
                    oproj(dqb, ot_tiles[dqb])
    nc.finalize()
    return nc


_CACHE = {}


def _get_runner():
    """Compile once per process; return f(in_maps) -> list of out dicts."""
    if "runner" in _CACHE:
        return _CACHE["runner"]
    import jax
    from jax.sharding import Mesh, PartitionSpec
    from jax.experimental.shard_map import shard_map
    from concourse import bass2jax

    nc = build_nc()
    bass2jax.install_neuronx_cc_hook()
    in_names, out_names, out_avals, zero_shapes = [], [], [], []
    for alloc in nc.m.functions[0].allocations:
        if not isinstance(alloc, mybir.MemoryLocationSet):
            continue
        name = alloc.memorylocations[0].name
        if alloc.kind == "ExternalInput":
            if name != "partition_id":
                in_names.append(name)
        elif alloc.kind == "ExternalOutput":
            out_names.append(name)
            shape = tuple(alloc.tensor_shape)
            dtype = mybir.dt.np(alloc.dtype)
            out_avals.append(jax.core.ShapedArray(shape, dtype))
            zero_shapes.append((shape, dtype))
    n_params = len(in_names)
    all_names = tuple(in_names + out_names)
    donate = tuple(range(n_params, n_params + len(out_names)))
    has_pid = nc.partition_id_tensor is not None

    def _body(*args):
        operands = list(args)
        names = all_names
        if has_pid:
            operands.append(bass2jax.partition_id_tensor())
            names = all_names + ("partition_id",)
        outs = bass2jax._bass_exec_p.bind(
            *operands, out_avals=tuple(out_avals), in_names=names,
            out_names=tuple(out_names), lowering_input_output_aliases=(),
            sim_require_finite=False, sim_require_nnan=False, nc=nc)
        return tuple(outs)

    devices = jax.devices()[:8]
    mesh = Mesh(np.asarray(devices), ("core",))
    specs = (PartitionSpec("core"),) * (n_params + len(out_names))
    f = jax.jit(shard_map(_body, mesh=mesh, in_specs=specs,
                          out_specs=(PartitionSpec("core"),) * len(out_names),
                          check_rep=False),
                donate_argnums=donate, keep_unused=True)

    def run(in_maps):
        concat_in = [np.concatenate([m[n] for m in in_maps], axis=0)
                     for n in in_names]
        concat_zeros = [np.zeros((8 * s[0], *s[1:]), d) for s, d in zero_shapes]
        outs = f(*concat_in, *concat_zeros)
        res = []
        for c in range(8):
            res.append({name: np.asarray(outs[i]).reshape(8, *out_avals[i].shape)[c]
                        for i, name in enumerate(out_names)})
        return res

    _CACHE["runner"] = run
    _CACHE["nc"] = nc
    return run


def make_in_maps(k, q, v, Wk, bk, Wq, bq, Wv, bv, Wo, bo):
    in_maps = []
    for c in range(8):
        b, g = divmod(c, 2)
        gs, ge = g * G, (g + 1) * G
        bqs = (bq[gs:ge] * SCALE).reshape(NCOUT, 128).T
        in_maps.append({
            "xq": np.ascontiguousarray(q[b].T).astype(ml_dtypes.bfloat16),
            "xk": np.ascontiguousarray(k[b].T).astype(ml_dtypes.bfloat16),
            "xv": np.ascontiguousarray(v[b].T).astype(ml_dtypes.bfloat16),
            "wq": np.ascontiguousarray(Wq[gs:ge, :].T).astype(ml_dtypes.bfloat16),
            "wk": np.ascontiguousarray(Wk[gs:ge, :].T).astype(ml_dtypes.bfloat16),
            "wv": np.ascontiguousarray(Wv[gs:ge, :].T).astype(ml_dtypes.bfloat16),
            "wo": np.ascontiguousarray(Wo[:, gs:ge].T).astype(ml_dtypes.bfloat16),
            "bq": np.ascontiguousarray(bqs, dtype=np.float32),
        })
    return in_maps


def kernel(k, q, v, Wk, bk, Wq, bq, Wv, bv, Wo, bo):
    k = np.asarray(k, dtype=np.float32)
    q = np.asarray(q, dtype=np.float32)
    v = np.asarray(v, dtype=np.float32)
    Wk, bk = np.asarray(Wk, np.float32), np.asarray(bk, np.float32)
    Wq, bq = np.asarray(Wq, np.float32), np.asarray(bq, np.float32)
    Wv, bv = np.asarray(Wv, np.float32), np.asarray(bv, np.float32)
    Wo, bo = np.asarray(Wo, np.float32), np.asarray(bo, np.float32)

    in_maps = make_in_maps(k, q, v, Wk, bk, Wq, bq, Wv, bv, Wo, bo)
    run = _get_runner()
    res = run(in_maps)
    host_bias = (bo + Wo @ bv).astype(np.float32)
    out = np.empty((B, T, C), np.float32)
    for b in range(B):
        out[b] = res[2 * b]["out"] + res[2 * b + 1]["out"] + host_bias[None, :]
    return out
